# revision 1
# baseline (speedup 1.0000x reference)
"""Bidirectional GRU layer for Trainium2, 8 NeuronCores.

Distribution: the two directions are independent GRUs over the same x.
Cores 0-3 run the forward direction on batch slices of 8; cores 4-7 run
the backward direction (host passes time-reversed x, so the device
kernel is identical). Weights are replicated per direction.

Device kernel (per core): unidirectional GRU, T=2048, B=8, I=H=512, in
"transposed" layout (feature dim on partitions):
  - input projections G^T = Wcat^T @ x^T computed blockwise (64 steps)
    straight into SBUF, fused with the recurrence (no DRAM round-trip),
  - recurrence A^T = Hcat^T @ h^T via 48 weight-stationary [128,128]
    chunk matmuls per step; gates/elementwise on [128, 4, 8] tiles;
    sigmoid/tanh on ACT (same table set); h written straight into the
    y output staging tile, which doubles as the h history.
"""
import numpy as np

T, B, I, H = 2048, 32, 512, 512
NCORES = 8
CORES_PER_DIR = 4
BL = B // CORES_PER_DIR          # batch per core = 8
KC = I // 128                    # contraction chunks = 4
MC = 3 * H // 128                # gate-row chunks = 12
S = 64                           # time steps per block
NBLK = T // S
G4 = KC * BL                     # packed h/gate tile width = 32

_cache = {}


def _legalize_waits(nc, max_waits=1):
    """The TRN2 walrus codegen here rejects instructions with more than one
    semaphore wait. Engine sequencers dispatch in order and sem-waits gate
    dispatch, so moving all-but-one wait onto NoOps inserted immediately
    before the offender is semantics-preserving."""
    import concourse.mybir as mybir

    ctr = 0
    for fn in nc.m.functions:
        for blk in fn.blocks:
            if not any(
                i.sync_info is not None and len(i.sync_info.on_wait) > max_waits
                for i in blk.instructions
            ):
                continue
            out = []
            for inst in blk.instructions:
                si = inst.sync_info
                if si is not None and len(si.on_wait) > max_waits:
                    waits = list(si.on_wait)
                    extra, keep = waits[:-max_waits], waits[-max_waits:]
                    for i in range(0, len(extra), max_waits):
                        nop = mybir.InstNoOp(name=f"lgw-{ctr}", ins=[], outs=[])
                        ctr += 1
                        nop.engine = inst.engine
                        nop.sync_info = mybir.SyncInfo(
                            on_wait=extra[i : i + max_waits], on_update=[]
                        )
                        nop.bass_nofuse = True
                        out.append(nop)
                    inst.sync_info = mybir.SyncInfo(
                        on_wait=keep, on_update=list(si.on_update)
                    )
                out.append(inst)
            blk.instructions = out


def _build_nc(static_blocks=None, use_bf16=False, s_blk=S, repeat=1):
    import concourse.bass as bass
    import concourse.mybir as mybir
    import concourse.tile as tile
    from concourse.bass import ds

    f32 = mybir.dt.float32
    bf16 = mybir.dt.bfloat16
    hdt = bf16 if use_bf16 else f32
    SB = s_blk
    NB = T // SB
    nc = bass.Bass()
    xT = nc.dram_tensor("xT", (I, T * BL), f32, kind="ExternalInput")
    wcat = nc.dram_tensor("wcat", (I, 3 * H), f32, kind="ExternalInput")
    hcat = nc.dram_tensor("hcat", (H, 3 * H), hdt, kind="ExternalInput")
    gbias = nc.dram_tensor("gbias", (MC, 128), f32, kind="ExternalInput")
    bnhb = nc.dram_tensor("bnhb", (128, G4), f32, kind="ExternalInput")
    yT = nc.dram_tensor("yT", (KC, 128, T, BL), f32, kind="ExternalOutput")

    xT_v = xT[:].rearrange("(k p) n -> p k n", p=128)
    wcat_v = wcat[:].rearrange("(k p) m -> p k m", p=128)
    hcat_v = hcat[:].rearrange("(k p) m -> p k m", p=128)
    gbias_v = gbias[:].rearrange("m p -> p m", p=128)
    yT_v = yT[:].rearrange("k p t b -> p k t b", p=128)

    Sig = mybir.ActivationFunctionType.Sigmoid
    Tanh = mybir.ActivationFunctionType.Tanh

    import contextlib

    class _StaticLoop(contextlib.AbstractContextManager):
        def __init__(self, i):
            self.i = i
        def __exit__(self, *a):
            return None

    with tile.TileContext(nc) as tc:
        with (
            tc.tile_pool(name="const", bufs=1) as cpool,
            tc.tile_pool(name="xp", bufs=2) as xpool,
            tc.tile_pool(name="gp", bufs=1) as gpool,
            tc.tile_pool(name="yp", bufs=1) as ypool,
            tc.tile_pool(name="ew", bufs=3) as ewpool,
            tc.tile_pool(name="pproj", bufs=2, space="PSUM") as ppool,
            tc.tile_pool(name="prec", bufs=3, space="PSUM") as rpool,
        ):
            wc = cpool.tile([128, KC, 3 * H], f32)
            hc = cpool.tile([128, KC, 3 * H], hdt)
            gb = cpool.tile([128, MC], f32)
            bnh_t = cpool.tile([128, KC, BL], f32)
            h_prev = cpool.tile([128, KC, BL], hdt)

            nc.sync.dma_start(wc[:], wcat_v)
            nc.sync.dma_start(hc[:], hcat_v)
            nc.sync.dma_start(gb[:], gbias_v)
            nc.sync.dma_start(bnh_t[:], bnhb[:].rearrange("p (k b) -> p k b", k=KC))
            nc.vector.memset(h_prev[:], 0.0)

            rep_ctx = (
                tc.For_i(0, repeat, 1) if repeat > 1 else contextlib.nullcontext()
            )
            loop_iter = (
                range(static_blocks)
                if static_blocks is not None
                else [None]
            )
            with rep_ctx:
              for _ib_py in loop_iter:
               with (
                  _StaticLoop(_ib_py)
                  if static_blocks is not None
                  else tc.For_i(0, NB, 1, hint_engines=(mybir.EngineType.PE,))
               ) as ib_ctx:
                ib = _ib_py if static_blocks is not None else ib_ctx
                xb = xpool.tile([128, KC, SB * BL], f32)
                nc.sync.dma_start(xb[:], xT_v[:, :, ds(ib * (SB * BL), SB * BL)])

                gblk = gpool.tile([128, MC, SB, BL], f32)
                gblk_f = gblk[:].rearrange("p m s b -> p m (s b)")
                for m in range(MC):
                    ps = ppool.tile([128, SB * BL], f32, tag="proj")
                    for k in range(KC):
                        nc.tensor.matmul(
                            ps[:],
                            wc[:, k, 128 * m : 128 * (m + 1)],
                            xb[:, k, :],
                            start=(k == 0),
                            stop=(k == KC - 1),
                        )
                    nc.vector.tensor_scalar_add(
                        gblk_f[:, m, :], ps[:], gb[:, m : m + 1]
                    )

                yb = ypool.tile([128, KC, SB, BL], f32)
                h_bf_prev = [None]
                for s in range(SB):
                    def h_k(k, s=s):
                        if s == 0:
                            return h_prev[:, k, :]
                        if use_bf16:
                            return h_bf_prev[0][:, k, :]
                        return yb[:, k, s - 1, :]

                    h_full = h_prev[:] if s == 0 else yb[:, :, s - 1, :]

                    pg_rc = rpool.tile([128, 2, KC, BL], f32, tag="pgrc", name="pgrc")
                    pg_n = rpool.tile([128, KC, BL], f32, tag="pgn", name="pgn")
                    for g in range(3):
                        for q in range(KC):
                            m = 4 * g + q
                            out_ap = pg_rc[:, g, q, :] if g < 2 else pg_n[:, q, :]
                            for k in range(KC):
                                nc.tensor.matmul(
                                    out_ap,
                                    hc[:, k, 128 * m : 128 * (m + 1)],
                                    h_k(k),
                                    start=(k == 0),
                                    stop=(k == KC - 1),
                                )
                    g_rc = gblk[:, 0 : 2 * KC, s, :].rearrange(
                        "p (g k) b -> p g k b", g=2
                    )
                    g_n = gblk[:, 2 * KC : 3 * KC, s, :]

                    trc = ewpool.tile([128, 2, KC, BL], f32, tag="trc")
                    nc.vector.tensor_add(trc[:], pg_rc[:], g_rc)
                    src_ = ewpool.tile([128, 2, KC, BL], f32, tag="src")
                    nc.scalar.activation(src_[:], trc[:], Sig)

                    tn = ewpool.tile([128, KC, BL], f32, tag="tn")
                    nc.vector.tensor_add(tn[:], pg_n[:], bnh_t[:])
                    u = ewpool.tile([128, KC, BL], f32, tag="u")
                    nc.vector.tensor_mul(u[:], src_[:, 0], tn[:])
                    v = ewpool.tile([128, KC, BL], f32, tag="v")
                    nc.vector.tensor_add(v[:], u[:], g_n)
                    n_t = ewpool.tile([128, KC, BL], f32, tag="n")
                    nc.scalar.activation(n_t[:], v[:], Tanh)

                    d = ewpool.tile([128, KC, BL], f32, tag="d")
                    nc.vector.tensor_sub(d[:], h_full, n_t[:])
                    e = ewpool.tile([128, KC, BL], f32, tag="e")
                    nc.vector.tensor_mul(e[:], src_[:, 1], d[:])
                    nc.vector.tensor_add(yb[:, :, s, :], n_t[:], e[:])
                    if use_bf16:
                        h_bf = ewpool.tile([128, KC, BL], bf16, tag="hbf", name="hbf")
                        nc.vector.tensor_copy(h_bf[:], yb[:, :, s, :])
                        h_bf_prev[0] = h_bf

                nc.vector.tensor_copy(h_prev[:], yb[:, :, SB - 1, :])
                nc.sync.dma_start(yT_v[:, :, ds(ib * SB, SB), :], yb[:])

    _legalize_waits(nc)
    return nc


def _prep_core_inputs(x_dir, p):
    """x_dir: [T, B, I] (already time-flipped for bwd). p: params for the
    direction. Returns per-core input maps (one per batch slice)."""
    wcat = np.ascontiguousarray(
        np.concatenate([p["Wri"], p["Wci"], p["Wni"]], axis=1), dtype=np.float32
    )
    hcat = np.ascontiguousarray(
        np.concatenate([p["Wrh"], p["Wch"], p["Wnh"]], axis=1), dtype=np.float32
    )
    gbias = np.ascontiguousarray(
        np.concatenate([p["br"], p["bi"], p["bni"]]).reshape(MC, 128), np.float32
    )
    bnhb = np.ascontiguousarray(
        np.broadcast_to(
            p["bnh"].reshape(KC, 128).T[:, :, None], (128, KC, BL)
        ).reshape(128, G4),
        np.float32,
    )
    import ml_dtypes
    hcat = hcat.astype(ml_dtypes.bfloat16)
    maps = []
    for ci in range(CORES_PER_DIR):
        xs = x_dir[:, ci * BL : (ci + 1) * BL, :]  # [T, BL, I]
        xTc = np.ascontiguousarray(xs.reshape(T * BL, I).T, dtype=np.float32)
        maps.append(
            {"xT": xTc, "wcat": wcat, "hcat": hcat, "gbias": gbias, "bnhb": bnhb}
        )
    return maps


def kernel(**inputs):
    from concourse.bass_utils import run_bass_kernel_spmd

    if "nc" not in _cache:
        _cache["nc"] = _build_nc(use_bf16=True)
    nc = _cache["nc"]

    x = np.asarray(inputs["x"], dtype=np.float32)
    pf = {k[:-2]: np.asarray(v, np.float32) for k, v in inputs.items() if k.endswith("_f")}
    pb = {k[:-2]: np.asarray(v, np.float32) for k, v in inputs.items() if k.endswith("_b")}

    x_rev = np.ascontiguousarray(x[::-1])
    in_maps = _prep_core_inputs(x, pf) + _prep_core_inputs(x_rev, pb)

    res = run_bass_kernel_spmd(nc, in_maps, core_ids=list(range(NCORES)))
    _cache["last_result"] = res

    y = np.empty((T, B, 2 * H), dtype=np.float32)
    for c in range(NCORES):
        yTc = res.results[c]["yT"]  # [KC, 128, T, BL]
        ys = np.transpose(yTc, (2, 3, 0, 1)).reshape(T, BL, H)
        d = c // CORES_PER_DIR
        ci = c % CORES_PER_DIR
        if d == 0:
            y[:, ci * BL : (ci + 1) * BL, :H] = ys
        else:
            y[:, ci * BL : (ci + 1) * BL, H:] = ys[::-1]
    return y



# revision 2
# speedup vs baseline: 2.9113x; 2.9113x over previous
"""Bidirectional GRU layer for Trainium2, 8 NeuronCores — v2.

Distribution: cores 0-3 forward direction on batch slices of 8; cores 4-7
backward (host passes time-reversed x). Weights replicated per direction.

Per-core device kernel (T=2048, BL=8, I=H=512), bf16 storage throughout:
  - input projections G = Wcat^T x + bias per block of 64 steps (bf16
    streams, 4x faster than the old fp32), biases via tensor_scalar_add.
  - recurrence per step, gate m-chunk order r(0-3), n(8-11), c(4-7):
    [128,128] bf16 weight chunks, N=8; bnh folded into the n-gate PSUM
    group as a K=1 matmul against a constant ones row.
  - chain: ar=pg_r+g_r (DVE); sr=Sig(ar) (ACT); u=sr*pg_n; v=u+g_n (DVE);
    n=Tanh(v) (ACT); ac=pg_c+g_c (DVE); sc=Sig(ac) (ACT);
    d=h-n; e=sc*d; y=n+e -> bf16 straight into the y/h-history tile.
"""
import numpy as np

T, B, I, H = 2048, 32, 512, 512
NCORES = 8
CORES_PER_DIR = 4
BL = B // CORES_PER_DIR          # batch per core = 8
KC = I // 128                    # contraction chunks = 4
MC = 3 * H // 128                # gate-row chunks = 12
S = 64                           # time steps per block
NBLK = T // S

_cache = {}


def _legalize_waits(nc, max_waits=1):
    """The TRN2 walrus codegen rejects instructions with more than one
    semaphore wait; move extra waits onto NoOps dispatched just before.
    First drop same-engine waits (engine queues are in-order, so a wait on
    the engine's own semaphore is satisfied by dispatch order) — this
    eliminates most NoOps, which otherwise sit in the dependency path."""
    import concourse.mybir as mybir

    eng_sem_prefix = {
        mybir.EngineType.PE: "PE_",
        mybir.EngineType.DVE: "DVE_",
        mybir.EngineType.Activation: "Activation_",
        mybir.EngineType.Pool: "Pool_",
        mybir.EngineType.SP: "SP_",
    }
    for fn in nc.m.functions:
        for blk in fn.blocks:
            for inst in blk.instructions:
                si = inst.sync_info
                if si is None or not si.on_wait or inst.engine is None:
                    continue
                pfx = eng_sem_prefix.get(inst.engine)
                if pfx is None:
                    continue
                kept = [
                    w for w in si.on_wait
                    if not (getattr(w, "ant_name", "") or "").startswith(pfx)
                ]
                if len(kept) != len(si.on_wait):
                    inst.sync_info = mybir.SyncInfo(
                        on_wait=kept, on_update=list(si.on_update)
                    )

    ctr = 0
    for fn in nc.m.functions:
        for blk in fn.blocks:
            if not any(
                i.sync_info is not None and len(i.sync_info.on_wait) > max_waits
                for i in blk.instructions
            ):
                continue
            out = []
            for inst in blk.instructions:
                si = inst.sync_info
                if si is not None and len(si.on_wait) > max_waits:
                    waits = list(si.on_wait)
                    extra, keep = waits[:-max_waits], waits[-max_waits:]
                    for i in range(0, len(extra), max_waits):
                        nop = mybir.InstNoOp(name=f"lgw-{ctr}", ins=[], outs=[])
                        ctr += 1
                        nop.engine = inst.engine
                        nop.sync_info = mybir.SyncInfo(
                            on_wait=extra[i : i + max_waits], on_update=[]
                        )
                        nop.bass_nofuse = True
                        out.append(nop)
                    inst.sync_info = mybir.SyncInfo(
                        on_wait=keep, on_update=list(si.on_update)
                    )
                out.append(inst)
            blk.instructions = out


def _build_nc(repeat=1, s_blk=S, nblk=None):
    import contextlib
    import concourse.bass as bass
    import concourse.mybir as mybir
    import concourse.tile as tile
    from concourse.bass import ds

    f32 = mybir.dt.float32
    bf16 = mybir.dt.bfloat16
    SB = s_blk
    NB = NBLK if nblk is None else nblk
    Tl = NB * SB
    Sig = mybir.ActivationFunctionType.Sigmoid
    Tanh = mybir.ActivationFunctionType.Tanh

    nc = bass.Bass()
    xT = nc.dram_tensor("xT", (I, Tl * BL), bf16, kind="ExternalInput")
    wcat = nc.dram_tensor("wcat", (I, 3 * H), bf16, kind="ExternalInput")
    hcat = nc.dram_tensor("hcat", (H, 3 * H), bf16, kind="ExternalInput")
    gbias = nc.dram_tensor("gbias", (MC, 128), f32, kind="ExternalInput")
    bnhw = nc.dram_tensor("bnhw", (1, H), bf16, kind="ExternalInput")
    yT = nc.dram_tensor("yT", (KC, 128, Tl, BL), bf16, kind="ExternalOutput")

    xT_v = xT[:].rearrange("(k p) n -> p k n", p=128)
    wcat_v = wcat[:].rearrange("(k p) m -> p k m", p=128)
    hcat_v = hcat[:].rearrange("(k p) m -> p k m", p=128)
    gbias_v = gbias[:].rearrange("m p -> p m", p=128)
    yT_v = yT[:].rearrange("k p t b -> p k t b", p=128)

    MR = list(range(0, 4))        # r gate m-chunks
    MCc = list(range(4, 8))       # c gate m-chunks
    MN = list(range(8, 12))       # n gate m-chunks

    with tile.TileContext(nc) as tc:
        with (
            tc.tile_pool(name="const", bufs=1) as cpool,
            tc.tile_pool(name="xp", bufs=2) as xpool,
            tc.tile_pool(name="gp", bufs=1) as gpool,
            tc.tile_pool(name="yp", bufs=2) as ypool,
            tc.tile_pool(name="ew", bufs=3) as ewpool,
            tc.tile_pool(name="pproj", bufs=2, space="PSUM") as ppool,
            tc.tile_pool(name="prec", bufs=2, space="PSUM") as rpool,
        ):
            wc = cpool.tile([128, KC, 3 * H], bf16)
            hc = cpool.tile([128, KC, 3 * H], bf16)
            gb = cpool.tile([128, MC], f32)
            bnh_w = cpool.tile([1, KC, 128], bf16)
            ones8 = cpool.tile([1, BL], bf16)
            # ping-pong h tiles: step s reads h_pp[s%2], writes h_pp[(s+1)%2]
            # (SB even, so parity is consistent across blocks); y additionally
            # lands in yb via an off-critical-path Pool copy so the matmuls
            # never read the DMA-bound yb tile (avoids a tile-granular WAR
            # that would serialize the chain behind the whole mm phase).
            h_pp0 = cpool.tile([128, KC, BL], bf16)
            h_pp1 = cpool.tile([128, KC, BL], bf16)
            h_pp = [h_pp0, h_pp1]

            nc.sync.dma_start(wc[:], wcat_v)
            nc.sync.dma_start(hc[:], hcat_v)
            nc.sync.dma_start(gb[:], gbias_v)
            nc.sync.dma_start(
                bnh_w[:], bnhw[:].rearrange("o (k p) -> o k p", k=KC)
            )
            nc.vector.memset(ones8[:], 1.0)
            nc.vector.memset(h_pp0[:], 0.0)

            rep_ctx = (
                tc.For_i(0, repeat, 1) if repeat > 1 else contextlib.nullcontext()
            )
            with rep_ctx:
              with tc.For_i(
                  0, NB, 1, hint_engines=(mybir.EngineType.PE,)
              ) as ib:
                xb = xpool.tile([128, KC, SB * BL], bf16, tag="xb")
                nc.sync.dma_start(xb[:], xT_v[:, :, ds(ib * (SB * BL), SB * BL)])

                gblk = gpool.tile([128, MC, SB, BL], bf16, tag="gblk")
                gblk_f = gblk[:].rearrange("p m s b -> p m (s b)")
                for m in range(MC):
                    ps = ppool.tile([128, SB * BL], f32, tag="proj")
                    for k in range(KC):
                        nc.tensor.matmul(
                            ps[:],
                            wc[:, k, 128 * m : 128 * (m + 1)],
                            xb[:, k, :],
                            start=(k == 0),
                            stop=(k == KC - 1),
                        )
                    nc.vector.tensor_scalar_add(
                        gblk_f[:, m, :], ps[:], gb[:, m : m + 1]
                    )

                yb = ypool.tile([128, KC, SB, BL], bf16, tag="yb")

                for s in range(SB):
                    h_ap = h_pp[s % 2][:]
                    h_out = h_pp[(s + 1) % 2]

                    pg_r = rpool.tile([128, KC, BL], f32, tag="pgr")
                    pg_n = rpool.tile([128, KC, BL], f32, tag="pgn")
                    pg_c = rpool.tile([128, KC, BL], f32, tag="pgc")

                    def rec_group(out_ap, m, close):
                        for k in range(KC):
                            nc.tensor.matmul(
                                out_ap,
                                hc[:, k, 128 * m : 128 * (m + 1)],
                                h_ap[:, k, :],
                                start=(k == 0),
                                stop=(close and k == KC - 1),
                            )

                    # r gates first (longest consumer chain)
                    for q, m in enumerate(MR):
                        rec_group(pg_r[:, q, :], m, close=True)
                    # n gates second; bnh folded in as a K=1 matmul
                    for q, m in enumerate(MN):
                        rec_group(pg_n[:, q, :], m, close=False)
                        nc.tensor.matmul(
                            pg_n[:, q, :],
                            bnh_w[:, q, :],
                            ones8[:],
                            start=False,
                            stop=True,
                        )
                    # c gates last (their consumers are needed latest)
                    for q, m in enumerate(MCc):
                        rec_group(pg_c[:, q, :], m, close=True)

                    g_r = gblk[:, 0:KC, s, :]
                    g_c = gblk[:, KC : 2 * KC, s, :]
                    g_n = gblk[:, 2 * KC : 3 * KC, s, :]

                    ar = ewpool.tile([128, KC, BL], f32, tag="ar")
                    nc.vector.tensor_add(ar[:], pg_r[:], g_r)
                    sr = ewpool.tile([128, KC, BL], f32, tag="sr")
                    nc.scalar.activation(sr[:], ar[:], Sig)

                    u = ewpool.tile([128, KC, BL], f32, tag="u")
                    nc.vector.tensor_mul(u[:], sr[:], pg_n[:])
                    v = ewpool.tile([128, KC, BL], f32, tag="v")
                    nc.vector.tensor_add(v[:], u[:], g_n)
                    n_t = ewpool.tile([128, KC, BL], f32, tag="n")
                    nc.scalar.activation(n_t[:], v[:], Tanh)

                    # ac as a bypass-STT with a value-neutral scalar read of v:
                    # gives the Tile scheduler (whose cost model ignores
                    # weight-load time and so thinks the c-gate matmuls finish
                    # "early") a true dependence that keeps ac AFTER u/v on the
                    # in-order DVE queue. Without it, ac lands at the queue
                    # head waiting on the c-gate matmuls and blocks the chain
                    # behind the whole mm phase.
                    ac = ewpool.tile([128, KC, BL], f32, tag="ac")
                    nc.vector.scalar_tensor_tensor(
                        ac[:],
                        pg_c[:],
                        v[:, 0, 0:1],
                        g_c,
                        mybir.AluOpType.bypass,
                        mybir.AluOpType.add,
                    )
                    sc = ewpool.tile([128, KC, BL], f32, tag="sc")
                    nc.scalar.activation(sc[:], ac[:], Sig)

                    d = ewpool.tile([128, KC, BL], f32, tag="d")
                    nc.vector.tensor_sub(d[:], h_ap, n_t[:])
                    e = ewpool.tile([128, KC, BL], f32, tag="e")
                    nc.vector.tensor_mul(e[:], sc[:], d[:])
                    nc.vector.tensor_add(h_out[:], n_t[:], e[:])
                    nc.gpsimd.tensor_copy(yb[:, :, s, :], h_out[:])

                nc.sync.dma_start(yT_v[:, :, ds(ib * SB, SB), :], yb[:])

    _legalize_waits(nc)
    return nc


def _prep_core_inputs(x_dir, p, s_blk=S, nblk=None):
    """x_dir: [Tl, B, I] (already time-flipped for bwd). Returns per-core
    input maps (one per batch slice)."""
    import ml_dtypes

    bf16 = ml_dtypes.bfloat16
    SB = s_blk
    NB = NBLK if nblk is None else nblk
    Tl = NB * SB
    wcat = np.concatenate([p["Wri"], p["Wci"], p["Wni"]], axis=1).astype(bf16)
    hcat = np.concatenate([p["Wrh"], p["Wch"], p["Wnh"]], axis=1).astype(bf16)
    gbias = np.ascontiguousarray(
        np.concatenate([p["br"], p["bi"], p["bni"]]).reshape(MC, 128), np.float32
    )
    bnhw = np.ascontiguousarray(p["bnh"].reshape(1, H)).astype(bf16)
    maps = []
    for ci in range(CORES_PER_DIR):
        xs = x_dir[:Tl, ci * BL : (ci + 1) * BL, :]     # [Tl, BL, I]
        xTc = np.ascontiguousarray(xs.reshape(Tl * BL, I).T.astype(bf16))
        maps.append(
            {
                "xT": xTc,
                "wcat": np.ascontiguousarray(wcat),
                "hcat": np.ascontiguousarray(hcat),
                "gbias": gbias,
                "bnhw": bnhw,
            }
        )
    return maps


def kernel(**inputs):
    from concourse.bass_utils import run_bass_kernel_spmd

    if "nc" not in _cache:
        _cache["nc"] = _build_nc()
    nc = _cache["nc"]

    x = np.asarray(inputs["x"], dtype=np.float32)
    pf = {k[:-2]: np.asarray(v, np.float32) for k, v in inputs.items() if k.endswith("_f")}
    pb = {k[:-2]: np.asarray(v, np.float32) for k, v in inputs.items() if k.endswith("_b")}

    x_rev = np.ascontiguousarray(x[::-1])
    in_maps = _prep_core_inputs(x, pf) + _prep_core_inputs(x_rev, pb)

    res = run_bass_kernel_spmd(nc, in_maps, core_ids=list(range(NCORES)))
    _cache["last_result"] = res

    y = np.empty((T, B, 2 * H), dtype=np.float32)
    for c in range(NCORES):
        yTc = np.asarray(res.results[c]["yT"], dtype=np.float32)  # [KC,128,T,BL]
        ys = np.transpose(yTc, (2, 3, 0, 1)).reshape(T, BL, H)
        d = c // CORES_PER_DIR
        ci = c % CORES_PER_DIR
        if d == 0:
            y[:, ci * BL : (ci + 1) * BL, :H] = ys
        else:
            y[:, ci * BL : (ci + 1) * BL, H:] = ys[::-1]
    return y
